# revision 45
# baseline (speedup 1.0000x reference)
"""Two-layer GAT (PyG GATConv semantics) as a Bass/Tile kernel on 8 TRN2 NeuronCores.

Strategy (graph/data parallel, dst-sharded):
  - Nodes padded to NPAD=50176, split into 8 contiguous shards of SHARD=6272
    (= 49 blocks x 128). Core k owns dst nodes [k*SHARD, (k+1)*SHARD).
  - Edges (incl. self loops) bucketed by dst block, sorted by dst, and split
    into two streams by table row (gather indices are int16, so each gather
    table half must stay under 32768 rows). Layers get separate stream plans:
    L1 splits at NPAD/2 over the node-major tab1; L2 splits at the AllGather
    group boundary over the group-permuted tab2a/tab2b.
  - P1: every core redundantly computes the full projection table
    tab1[n] = [h(n) bf16(384, ch-major) | a_src(n) f32(6) | a_dst(n) f32(6)]
    (1024B rows), h = x@W1, a_* = h . att_* via folded weight columns.
    4-block macro-tiles amortize HWDGE fixed costs; h is stored ch-major
    (col = ch*H + h) so the edge phase's ex*h multiply runs in 2x DVE mode.
  - P2 (edge phase, 4-stage software pipeline so no in-order engine stream
    ever queues behind a cross-engine dependency):
      s1: dma_gather of tab rows for the block's in-edges; per-chunk one-hot
          U[e,d] built by tensor_scalar is_equal (4x DVE); PE transposes in
          packs of 4 + one ACT copy -> UT; per-edge a_dst via tiny PE matmuls
          UT.T @ a_dst_blk; logits + fused leaky-relu + Exp.
      s2: L1 scales gathered h by ex in place (2x DVE) and overwrites the
          consumed a_src cols with ex; L2 rebuilds the one-hot scaled by ex
          (fused is_equal*mult). pnum = U.T @ rows accumulated in PSUM.
      s3a: segment-softmax division (+bias, ReLU).
      s3b (L1): h1 transpose, P3 matmul h2e = relu(h1) @ [W2|W2@att2], h2e
          staging; the AllGather is split in two groups: group 1 (32 blocks)
          overlaps the tail of the L1 edge phase, group 2 (17 blocks) follows.
  - P4: same edge machinery for layer 2 (1 head, 32 ch) -> y (own shard).
    tab2 rows are [h2 32 | one | pad | a_src2 f32 | a_dst2 f32] bf16; the
    ones column folds the softmax denominator into the same matmul.

No barriers: Tile's DRAM shadow-memory dependency tracking orders the tab1
writes vs gathers and the h2e writes vs collectives vs tab2 gathers.

kernel() takes full inputs, preprocesses indices on the host (sorting /
bucketing / layout only), compiles one SPMD NEFF, runs it on cores 0..7 via
bass_utils.run_bass_kernel_spmd, and concatenates the per-core outputs.
"""

import contextlib
import dataclasses

import numpy as np

import concourse.bass as bass
import concourse.mybir as mybir
import concourse.tile as tile
import concourse.bacc as bacc
from concourse.bass_utils import run_bass_kernel_spmd
from concourse.alu_op_type import AluOpType

F32 = mybir.dt.float32
BF16 = mybir.dt.bfloat16
I16 = mybir.dt.int16

PAD_OFF = 200.0  # dst_off sentinel for padding edges -> one-hot column all-zero


@dataclasses.dataclass
class Cfg:
    N: int = 50000
    E: int = 800000
    IN: int = 256
    HEADS: int = 6
    HID: int = 64
    OUT: int = 32
    NEG: float = 0.2
    NC: int = 8
    NB: int = 49
    BLK: int = 128
    GRP: int = 32           # blocks per core in AllGather group 1 (of NB)
    neg_pad: bool = False   # -1 gather indices (hangs the gather ucode)
    skip_cc: bool = False   # debug: replace AllGather with local copy (wrong results)

    @property
    def D1(self):
        return self.HEADS * self.HID

    @property
    def SHARD(self):
        return self.NB * self.BLK

    @property
    def NPAD(self):
        return self.NC * self.SHARD

    @property
    def HALF(self):
        return self.NPAD // 2

    @property
    def ROW1(self):
        need = self.D1 // 2 + 2 * self.HEADS
        return ((need + 63) // 64) * 64          # f32 elems / tab1 row

    @property
    def ROW2(self):
        # [h2 32 | one 1 | pad 1 | asrc2 f32 | adst2 f32 | pad] bf16 elems
        return 128


def _wrap_idx(idx_flat):
    """int16 gather index layout: index i at [partition i%16, free i//16],
    replicated down to 128 partitions."""
    n = idx_flat.shape[0]
    assert n % 16 == 0
    w = idx_flat.reshape(-1, 16).T.astype(np.int16)
    return np.tile(w, (8, 1))


@dataclasses.dataclass
class LayerPlan:
    chA: list
    chB: list
    G: int
    idx: np.ndarray        # [NC, 128, totw] int16 wrapped gather indices
    dcol: np.ndarray       # [NC, 128, G] f32 per-chunk dst offsets
    totw: int


@dataclasses.dataclass
class EdgePlan:
    l1: LayerPlan
    l2: LayerPlan


def build_edge_plan(cfg: Cfg, edge_index: np.ndarray) -> EdgePlan:
    c = cfg
    N, NC, NB, BLK = c.N, c.NC, c.NB, c.BLK
    SHARD, HALF = c.SHARD, c.HALF
    src = np.concatenate([np.asarray(edge_index[0], np.int64),
                          np.arange(N, dtype=np.int64)])
    dst = np.concatenate([np.asarray(edge_index[1], np.int64),
                          np.arange(N, dtype=np.int64)])
    core = dst // SHARD
    blk = (dst % SHARD) // BLK
    off = (dst % BLK).astype(np.float32)

    def rowmap1(n):
        return n

    G1R = c.GRP * BLK                     # rows per core in AllGather group 1

    def rowmap2(n):
        # tab2 layout: two AllGather groups, each rank-major inside
        q = n // SHARD
        r = n % SHARD
        return np.where(r < G1R, q * G1R + r,
                        NC * G1R + q * (SHARD - G1R) + (r - G1R))

    def layer(rowmap, half, self_rows=1):
        row = rowmap(src)
        strm = (row >= half).astype(np.int64)
        lsrc = (row - strm * half).astype(np.int32)
        slot = (core * NB + blk) * 2 + strm
        order = np.argsort(slot, kind="stable")
        slot_s, lsrc_s, off_s = slot[order], lsrc[order], off[order]
        counts = np.bincount(slot_s, minlength=NC * NB * 2)
        starts = np.concatenate([[0], np.cumsum(counts)])

        cnt = counts.reshape(NC, NB, 2)
        ch = np.maximum(1, -(-cnt.max(axis=0) // BLK))
        chA, chB = ch[:, 0].tolist(), ch[:, 1].tolist()
        G = int(sum(a + b for a, b in zip(chA, chB)))
        sr = self_rows
        totw = sum((sr + a + sr + b) * (BLK // 16)
                   for a, b in zip(chA, chB))

        idx_all = np.zeros((NC, 128, totw), np.int16)
        dcol = np.full((NC, 128, G), PAD_OFF, np.float32)
        for k in range(NC):
            wpos = 0
            g = 0
            for b in range(NB):
                own_rows = rowmap(k * SHARD + b * BLK + np.arange(BLK))
                s_own = int(own_rows[0] >= half)
                own_lsrc = (own_rows - s_own * half).astype(np.int32)
                for s, nch in ((0, chA[b]), (1, chB[b])):
                    seg = np.zeros(((sr + nch) * BLK,), np.int32)
                    if s == s_own and sr:
                        seg[:BLK] = own_lsrc
                    sidx = (k * NB + b) * 2 + s
                    st, en = starts[sidx], starts[sidx + 1]
                    cntk = en - st
                    assert cntk <= nch * BLK
                    seg[sr * BLK:sr * BLK + cntk] = lsrc_s[st:en]
                    w = _wrap_idx(seg)
                    idx_all[k][:, wpos:wpos + w.shape[1]] = w
                    wpos += w.shape[1]
                    offs = np.full((nch * BLK,), PAD_OFF, np.float32)
                    offs[:cntk] = off_s[st:en]
                    offs = offs.reshape(nch, BLK)
                    for cc in range(nch):
                        dcol[k][:, g] = offs[cc]
                        g += 1
            assert g == G and wpos == totw
        return LayerPlan(chA, chB, G, idx_all, dcol, totw)

    # L2 stream boundary = the AllGather group boundary, so each gather
    # stream reads one whole Shared tensor (offset collectives misbehave)
    assert c.NC * G1R <= 32768
    # L2 needs no self rows: a_dst2 of own blocks is computed on-core in P3
    return EdgePlan(layer(rowmap1, HALF),
                    layer(rowmap2, c.NC * G1R, self_rows=0))


def build_nc(cfg: Cfg, plan: EdgePlan):
    c = cfg
    nc = bacc.Bacc("TRN2", target_bir_lowering=False, debug=False,
                   enable_asserts=False, num_devices=c.NC,
                   num_swdge_queues=4, dynamic_dma_scratch_size=32768)

    H = c.HEADS
    D1, IN, OUT = c.D1, c.IN, c.OUT
    NBK = c.NPAD // 128
    KIN = IN // 128
    K1 = D1 // 128

    xt = nc.dram_tensor("xt", [IN, c.NPAD], BF16, kind="ExternalInput")
    w1 = nc.dram_tensor("w1", [IN, D1], BF16, kind="ExternalInput")
    w1t = nc.dram_tensor("w1t", [D1, IN], BF16, kind="ExternalInput")
    attbd1 = nc.dram_tensor("attbd1", [D1, 2 * H], BF16, kind="ExternalInput")
    w2 = nc.dram_tensor("w2", [D1, OUT], BF16, kind="ExternalInput")
    w2t = nc.dram_tensor("w2t", [OUT, D1], BF16, kind="ExternalInput")
    att2 = nc.dram_tensor("att2", [OUT, 2], BF16, kind="ExternalInput")
    b1r = nc.dram_tensor("b1r", [128, D1], F32, kind="ExternalInput")
    b2r = nc.dram_tensor("b2r", [128, OUT], F32, kind="ExternalInput")
    iota_r = nc.dram_tensor("iota_r", [128, 128], BF16, kind="ExternalInput")
    iota_c = nc.dram_tensor("iota_c", [128, 1], F32, kind="ExternalInput")
    ident = nc.dram_tensor("ident", [128, 128], BF16, kind="ExternalInput")
    ind_a = nc.dram_tensor("ind_a", [128, 1], F32, kind="ExternalInput")
    ind_b = nc.dram_tensor("ind_b", [128, 1], F32, kind="ExternalInput")
    TOTW = plan.l1.totw + plan.l2.totw
    GTOT = plan.l1.G + plan.l2.G
    idx_d = nc.dram_tensor("idx_d", [128, TOTW], I16, kind="ExternalInput")
    dcol_d = nc.dram_tensor("dcol_d", [128, GTOT], F32, kind="ExternalInput")

    G1R = c.GRP * 128
    tab1 = nc.dram_tensor("tab1", [c.NPAD, c.ROW1], F32)
    h2e_a = nc.dram_tensor("h2e_a", [G1R, c.ROW2], BF16)
    h2e_b = nc.dram_tensor("h2e_b", [c.SHARD - G1R, c.ROW2], BF16)
    tab2a = nc.dram_tensor("tab2a", [c.NC * G1R, c.ROW2], BF16,
                           addr_space="Shared")
    tab2b = nc.dram_tensor("tab2b", [c.NPAD - c.NC * G1R, c.ROW2], BF16,
                           addr_space="Shared")
    y = nc.dram_tensor("y", [c.SHARD, OUT], F32, kind="ExternalOutput")

    AS0 = D1 // 2            # f32 col of a_src in a tab1 row
    AD0 = AS0 + H            # f32 col of a_dst in a tab1 row

    with tile.TileContext(nc, num_cores=c.NC) as tc:
        with contextlib.ExitStack() as ctx:
            consts = ctx.enter_context(tc.tile_pool(name="consts", bufs=1))
            projx = ctx.enter_context(tc.tile_pool(name="projx", bufs=4))
            stg = ctx.enter_context(tc.tile_pool(name="stg", bufs=4))
            gp = ctx.enter_context(tc.tile_pool(name="gath", bufs=4))
            up = ctx.enter_context(tc.tile_pool(name="upool", bufs=2))
            wk = ctx.enter_context(tc.tile_pool(name="wk", bufs=4))
            idxp = ctx.enter_context(tc.tile_pool(name="idxp", bufs=4))
            pmm = ctx.enter_context(tc.tile_pool(name="pmm", bufs=3, space="PSUM"))
            ppa = ctx.enter_context(tc.tile_pool(name="ppa", bufs=2, space="PSUM"))
            paux = ctx.enter_context(tc.tile_pool(name="paux", bufs=3, space="PSUM"))

            def load_const(dram, shape, dtype):
                t = consts.tile(shape, dtype, tag=dram.name)
                nc.sync.dma_start(t[:], dram.ap())
                return t

            iota_row = load_const(iota_r, [128, 128], BF16)
            iota_col = load_const(iota_c, [128, 1], F32)
            ident_b = load_const(ident, [128, 128], BF16)
            b1_sb = load_const(b1r, [128, D1], F32)
            b2_sb = load_const(b2r, [128, OUT], F32)
            indA = load_const(ind_a, [128, 1], F32)
            indB = load_const(ind_b, [128, 1], F32)
            dcol_sb = load_const(dcol_d, [128, GTOT], F32)
            # a_dst2 of own blocks, written during P3 (l1_out_b), so the L2
            # edge phase needs no self-row gathers at all
            adst2_all = consts.tile([128, c.NB], F32, tag="adst2_all")

            # ---- W1e [128, KIN, D1+2H] and W2e [128, K1, OUT+2] ----
            w1e = consts.tile([128, KIN, D1 + 2 * H], BF16, tag="w1e")
            for ki in range(KIN):
                nc.sync.dma_start(w1e[:, ki, 0:D1],
                                  w1.ap()[ki * 128:(ki + 1) * 128, :])
            w1t_s = consts.tile([128, K1, IN], BF16, tag="w1t_s")
            for kj in range(K1):
                nc.sync.dma_start(w1t_s[:, kj, :],
                                  w1t.ap()[kj * 128:(kj + 1) * 128, :])
            abd_s = consts.tile([128, K1, 2 * H], BF16, tag="abd_s")
            for kj in range(K1):
                nc.sync.dma_start(abd_s[:, kj, :],
                                  attbd1.ap()[kj * 128:(kj + 1) * 128, :])
            for ki in range(KIN):
                ps = paux.tile([128, 2 * H], F32, tag="aux")
                for kj in range(K1):
                    nc.tensor.matmul(ps[:], w1t_s[:, kj, ki * 128:(ki + 1) * 128],
                                     abd_s[:, kj, :], start=(kj == 0),
                                     stop=(kj == K1 - 1))
                nc.scalar.copy(w1e[:, ki, D1:D1 + 2 * H], ps[:])

            w2e = consts.tile([128, K1, OUT + 2], BF16, tag="w2e")
            for kj in range(K1):
                nc.sync.dma_start(w2e[:, kj, 0:OUT],
                                  w2.ap()[kj * 128:(kj + 1) * 128, :])
            w2t_s = consts.tile([128, D1], BF16, tag="w2t_s")
            nc.sync.dma_start(w2t_s[:OUT, :], w2t.ap())
            att2_s = consts.tile([128, 2], BF16, tag="att2_s")
            nc.sync.dma_start(att2_s[:OUT, :], att2.ap())
            for kj in range(K1):
                ps = paux.tile([128, 2], F32, tag="aux")
                nc.tensor.matmul(ps[:], w2t_s[:OUT, kj * 128:(kj + 1) * 128],
                                 att2_s[:OUT, :], start=True, stop=True)
                nc.scalar.copy(w2e[:, kj, OUT:OUT + 2], ps[:])

            # pre-zero gather buffers: pad slots skipped by the gather
            # must hold finite values for the ex*h multiply / pnum matmul
            mx1 = 1 + max(max(plan.l1.chA), max(plan.l1.chB))
            mx2 = max(max(plan.l2.chA), max(plan.l2.chB))
            for s in range(2):
                for i in range(4):
                    if i < 3:
                        z1 = gp.tile([128, mx1, c.ROW1], F32, tag=f"g1{s}",
                                     bufs=3)
                        nc.vector.memset(z1[:], 0.0)
                    z2 = gp.tile([128, mx2, c.ROW2], BF16, tag=f"g2{s}",
                                 bufs=4)
                    nc.vector.memset(z2[:], 0.0)

            # ---- P1: replicated projection -> tab1 ----
            # 4-block macro tiles: one xt load per KIN slice and one tab1
            # write per 4 blocks (HWDGE fixed cost dominates small DMAs)
            assert NBK % 4 == 0
            for nb4 in range(NBK // 4):
                xts = []
                for ki in range(KIN):
                    xtile = projx.tile([128, 512], BF16, tag="xt")
                    nc.sync.dma_start(
                        xtile[:], xt.ap()[ki * 128:(ki + 1) * 128,
                                          nb4 * 512:(nb4 + 1) * 512])
                    xts.append(xtile)
                st = stg.tile([128, 4, AD0 + H], F32, tag="stage1")
                for q in range(4):
                    nb = nb4 * 4 + q
                    ps = pmm.tile([128, D1 + 2 * H], F32, tag="mm")
                    for ki in range(KIN):
                        nc.tensor.matmul(
                            ps[:], xts[ki][:, q * 128:(q + 1) * 128],
                            w1e[:, ki, :], start=(ki == 0),
                            stop=(ki == KIN - 1))
                    # h block stored ch-major (col = ch*H + h) so the edge
                    # phase's ex*h multiply is 2x-mode eligible on DVE
                    stv = st[:, q, 0:AS0].bitcast(BF16).rearrange(
                        "p (ch h) -> p ch h", h=H)
                    psv = ps[:, 0:D1].rearrange("p (h ch) -> p ch h", h=H)
                    if q % 2 == 0:
                        nc.scalar.copy(stv, psv)
                    else:
                        nc.vector.tensor_copy(stv, psv)
                    nc.vector.tensor_copy(st[:, q, AS0:AD0 + H],
                                          ps[:, D1:D1 + 2 * H])
                # tab1 row pad cols are never read by the edge phase
                nc.sync.dma_start(
                    tab1.ap()[nb4 * 512:(nb4 + 1) * 512, 0:AD0 + H]
                    .rearrange("(q p) e -> p q e", p=128),
                    st[:])

            # ---- shared edge phase ----
            self_q = [0]

            def edge_phase(lp, wbase, gbase, tabv_a, tabv_b, row_elems,
                           row_dtype, nh, chans, as_col, ad_col, out_cb, tag,
                           l2_mode=False):
                # 3-stage software pipeline across dst blocks so the in-order
                # PE stream never queues behind a cross-engine dependency:
                #   s1: gathers + one-hots + transposes/copies + paE + logits
                #   s2: pnum accumulation matmuls
                #   s3: out_cb (division / staging)
                wpos = [wbase]
                gpos = [gbase]

                sr = 0 if l2_mode else 1
                pcm = 7  # >7-chunk pieces (>896 descs) hang the real ring

                def s1(b):
                    nA, nB_ = lp.chA[b], lp.chB[b]
                    nr = nA + nB_
                    g = gpos[0]
                    gpos[0] += nr
                    gts = []
                    niA = (sr + nA) * 8
                    niB = (sr + nB_) * 8
                    itb = idxp.tile([128, niA + niB], I16, tag=f"idx{tag}")
                    nc.sync.dma_start(
                        itb[:], idx_d.ap()[:, wpos[0]:wpos[0] + niA + niB])
                    wpos[0] += niA + niB
                    for s, nch in ((0, nA), (1, nB_)):
                        it = itb[:, 0:niA] if s == 0 else \
                            itb[:, niA:niA + niB]
                        gt = gp.tile([128, sr + nch, row_elems], row_dtype,
                                     tag=f"g{tag}{s}",
                                     bufs=3 if not l2_mode else 4)
                        # pieces of <= 7 chunks (896 descs)
                        po = 0
                        while po < sr + nch:
                            pc = min(pcm, sr + nch - po)
                            nc.gpsimd.dma_gather(
                                gt[:, po:po + pc, :],
                                tabv_a if s == 0 else tabv_b,
                                it[:, po * 8:(po + pc) * 8],
                                pc * 128, pc * 128, row_elems,
                                queue_num=self_q[0])
                            self_q[0] = (self_q[0] + 1) % 4
                            po += pc
                        gts.append(gt)
                    gA, gB = gts

                    adst_b = wk.tile([128, nh], BF16, tag=f"adstb{tag}")
                    if l2_mode:
                        # a_dst2 was stashed in SBUF during P3 of this block
                        nc.vector.tensor_copy(adst_b[:],
                                              adst2_all[:, b:b + 1])
                    else:
                        adst = wk.tile([128, nh], F32, tag=f"adst{tag}")
                        ga_a = gA[:, 0:1, :].bitcast(F32)[
                            :, 0, ad_col:ad_col + nh]
                        ga_b = gB[:, 0:1, :].bitcast(F32)[
                            :, 0, ad_col:ad_col + nh]
                        nc.vector.tensor_scalar(adst[:], ga_a, indA[:, 0:1],
                                                None, op0=AluOpType.mult)
                        tmpb = wk.tile([128, nh], F32, tag=f"adst2{tag}")
                        nc.vector.tensor_scalar(tmpb[:], ga_b, indB[:, 0:1],
                                                None, op0=AluOpType.mult)
                        nc.vector.tensor_tensor(adst[:], adst[:], tmpb[:],
                                                op=AluOpType.add)
                        nc.vector.tensor_copy(adst_b[:], adst[:])

                    # per-chunk one-hot build via tensor_scalar (4x DVE
                    # mode); transposes + PSUM->SBUF copies first (copies
                    # alternate ACT/DVE), then the paE matmuls, so the
                    # in-order PE stream never queues behind a copy
                    uall = up.tile([128, nr, 128], BF16, tag=f"ua{tag}")
                    uts = []
                    for r in range(nr):
                        nc.vector.tensor_scalar(
                            uall[:, r, :], iota_row[:],
                            dcol_sb[:, g + r:g + r + 1], None,
                            op0=AluOpType.is_equal)
                    # transposes in packs of 4 per PSUM tile so one ACT copy
                    # moves 4 one-hots (the per-op PSUM-access latency is the
                    # dominant copy cost)
                    for r0 in range(0, nr, 4):
                        pk = min(4, nr - r0)
                        pst = paux.tile([128, 4, 128], BF16, tag="aux")
                        for j in range(pk):
                            nc.tensor.transpose(pst[:, j, :],
                                                uall[:, r0 + j, :], ident_b[:])
                        UT = wk.tile([128, 4, 128], BF16, tag=f"UT{tag}")
                        nc.scalar.copy(UT[:, 0:pk, :], pst[:, 0:pk, :])
                        uts.append(UT)
                    paE = ppa.tile([128, nh * nr], F32, tag="pa")
                    for r in range(nr):
                        nc.tensor.matmul(paE[:, r * nh:(r + 1) * nh],
                                         uts[r // 4][:, r % 4, :], adst_b[:],
                                         start=True, stop=True)

                    esum = wk.tile([128, nh * nr], F32, tag=f"es{tag}")
                    for s, nch, base in ((0, nA, 0), (1, nB_, nA)):
                        if nch == 0:
                            continue
                        gt = gA if s == 0 else gB
                        asrc = gt[:, sr:sr + nch, :].bitcast(F32)[
                            :, :, as_col:as_col + nh]
                        pv = paE[:, base * nh:(base + nch) * nh].rearrange(
                            "p (ch h) -> p ch h", h=nh)
                        ev = esum[:, base * nh:(base + nch) * nh].rearrange(
                            "p (ch h) -> p ch h", h=nh)
                        nc.vector.tensor_tensor(ev, asrc, pv, op=AluOpType.add)
                    # leaky relu fused: max(x*neg, x)
                    lk = wk.tile([128, nh * nr], F32, tag=f"lk{tag}")
                    nc.vector.scalar_tensor_tensor(
                        lk[:], esum[:], c.NEG, esum[:],
                        op0=AluOpType.mult, op1=AluOpType.max)
                    # L2 consumes ex only as a tensor_scalar scalar2 (f32)
                    ex = wk.tile([128, nh * nr], F32 if l2_mode else BF16,
                                 tag=f"ex{tag}")
                    nc.scalar.activation(ex[:], lk[:],
                                         mybir.ActivationFunctionType.Exp)

                    return (gA, gB, uall, ex, g, nA, nB_)

                def s2(state):
                    # ex-dependent work lives here, one pipeline stage after
                    # s1, so the in-order DVE stream never stalls on exp
                    gA, gB, uall, ex, g, nA, nB_ = state
                    nr = nA + nB_
                    if l2_mode:
                        # rebuild the one-hot scaled by ex in place (fused
                        # is_equal + mult tensor_scalar, 4x DVE mode)
                        for r in range(nr):
                            nc.vector.tensor_scalar(
                                uall[:, r, :], iota_row[:],
                                dcol_sb[:, g + r:g + r + 1],
                                ex[:, r:r + 1],
                                op0=AluOpType.is_equal, op1=AluOpType.mult)
                    else:
                        # scale gathered h by ex in place (16-bit 2x DVE) and
                        # overwrite the consumed asrc bf16 cols with ex; the
                        # pnum matmul then reads the gather tile directly
                        for s, nch, base in ((0, nA, 0), (1, nB_, nA)):
                            gt = gA if s == 0 else gB
                            gb = gt[:, sr:sr + nch, :].bitcast(BF16)
                            hview = gb[:, :, 0:chans].rearrange(
                                "p c (ch h) -> p c ch h", h=nh)
                            exs = ex[:, base * nh:(base + nch) * nh]
                            exb = exs.rearrange(
                                "p (c o h) -> p c o h", h=nh, o=1,
                            ).broadcast_to([128, nch, chans // nh, nh])
                            nc.vector.tensor_tensor(hview, hview, exb,
                                                    op=AluOpType.mult)
                            nc.vector.tensor_copy(
                                gb[:, :, chans:chans + nh],
                                exs.rearrange("p (c h) -> p c h", h=nh))
                    npc = chans + (1 if l2_mode else nh)
                    pnum = pmm.tile([128, npc], F32, tag="mm")
                    for r in range(nr):
                        s = 0 if r < nA else 1
                        cpos = sr + (r if s == 0 else r - nA)
                        gt = gA if s == 0 else gB
                        if l2_mode:
                            rhs = gt[:, cpos, 0:chans + 1]
                        else:
                            rhs = gt[:, cpos:cpos + 1, :].bitcast(BF16)[
                                :, 0, 0:chans + nh]
                        nc.tensor.matmul(pnum[:], uall[:, r, :], rhs,
                                         start=(r == 0), stop=(r == nr - 1))
                    return pnum

                out_a, out_b = out_cb
                st1 = {}
                st2 = {}
                st3 = {}
                for b in range(c.NB + 3):
                    if b < c.NB:
                        st1[b] = s1(b)
                    if 0 <= b - 1 < c.NB:
                        st2[b - 1] = s2(st1.pop(b - 1))
                    if 0 <= b - 2 < c.NB:
                        st3[b - 2] = out_a(b - 2, st2.pop(b - 2))
                    if b - 3 >= 0:
                        out_b(b - 3, st3.pop(b - 3))

            # ---- P2: layer-1 edges (P3 h2e staging fused per block,
            # AllGather split into two groups so group 1 overlaps the tail
            # of the layer-1 edge phase; DRAM RAW deps order everything) ----
            tabA1 = tab1.ap()[0:c.HALF, :]
            tabB1 = tab1.ap()[c.HALF:c.NPAD, :]

            def emit_coll(src_t, dst_t):
                if c.skip_cc:
                    rows = src_t.shape[0]
                    for q in range(c.NC):
                        nc.sync.dma_start(
                            dst_t.ap()[q * rows:(q + 1) * rows, :],
                            src_t.ap())
                else:
                    nc.gpsimd.collective_compute(
                        "AllGather", AluOpType.bypass,
                        replica_groups=[list(range(c.NC))],
                        ins=[src_t.ap()], outs=[dst_t.ap()])

            def l1_out_a(b, pnum):
                den = wk.tile([128, H], F32, tag="den1")
                nc.vector.tensor_scalar(den[:], pnum[:, D1:D1 + H], 1e-30, None,
                                        op0=AluOpType.max)
                rec = wk.tile([128, H], F32, tag="rec1")
                nc.vector.reciprocal(rec[:], den[:])
                tmp = wk.tile([128, D1], F32, tag="tmp1")
                nv = pnum[:, 0:D1].rearrange("p (ch h) -> p ch h", h=H)
                rb = rec[:].rearrange("p (o h) -> p o h", o=1).broadcast_to(
                    [128, c.HID, H])
                tv = tmp[:].rearrange("p (ch h) -> p ch h", h=H)
                nc.vector.tensor_tensor(tv, nv, rb, op=AluOpType.mult)
                nc.vector.tensor_tensor(tmp[:], tmp[:], b1_sb[:],
                                        op=AluOpType.add)
                h1s = wk.tile([128, D1], BF16, tag="h1s")
                nc.scalar.activation(h1s[:], tmp[:],
                                     mybir.ActivationFunctionType.Relu)
                return h1s

            def l1_out_b(b, h1s):
                h1T = wk.tile([128, K1, 128], BF16, tag="h1T")
                for j in range(K1):
                    pst = paux.tile([128, 128], BF16, tag="aux")
                    nc.tensor.transpose(pst[:], h1s[:, j * 128:(j + 1) * 128],
                                        ident_b[:])
                    nc.scalar.copy(h1T[:, j, :], pst[:])
                # P3 for this block: tab2 row (bf16 cols) = [h2 0:32 | one 32
                # | pad 33 | asrc2 f32 at 34:36 | adst2 f32 at 36:38 | pad]
                ps = paux.tile([128, OUT + 2], F32, tag="aux")
                for kj in range(K1):
                    nc.tensor.matmul(ps[:], h1T[:, kj, :], w2e[:, kj, :],
                                     start=(kj == 0), stop=(kj == K1 - 1))
                nc.vector.tensor_copy(adst2_all[:, b:b + 1],
                                      ps[:, OUT + 1:OUT + 2])
                st2 = stg.tile([128, c.ROW2], BF16, tag="stage2")
                nc.vector.tensor_copy(st2[:, 0:OUT], ps[:, 0:OUT])
                nc.vector.memset(st2[:, OUT:OUT + 2], 1.0)
                nc.vector.tensor_copy(st2[:, OUT + 2:OUT + 6].bitcast(F32),
                                      ps[:, OUT:OUT + 2])
                if b < c.GRP:
                    dst = h2e_a.ap()[b * 128:(b + 1) * 128, 0:OUT + 6]
                else:
                    dst = h2e_b.ap()[(b - c.GRP) * 128:
                                     (b - c.GRP + 1) * 128, 0:OUT + 6]
                nc.sync.dma_start(dst, st2[:, 0:OUT + 6])
                if b == c.GRP - 1:
                    emit_coll(h2e_a, tab2a)

            edge_phase(plan.l1, 0, 0, tabA1, tabB1, c.ROW1, F32, H, D1,
                       AS0, AD0, (l1_out_a, l1_out_b), "1")
            emit_coll(h2e_b, tab2b)

            # ---- P4: layer-2 edges ----
            tabA2 = tab2a.ap()
            tabB2 = tab2b.ap()
            AS2 = (OUT + 2) // 2  # f32 col of a_src2 in a tab2 row

            def l2_out_a(b, pnum):
                den = wk.tile([128, 1], F32, tag="den2")
                nc.vector.tensor_scalar(den[:], pnum[:, chans2:chans2 + 1],
                                        1e-30, None, op0=AluOpType.max)
                rec = wk.tile([128, 1], F32, tag="rec2")
                nc.vector.reciprocal(rec[:], den[:])
                tmp = wk.tile([128, OUT], F32, tag="tmp2")
                nc.vector.tensor_scalar(tmp[:], pnum[:, 0:OUT], rec[:, 0:1],
                                        None, op0=AluOpType.mult)
                nc.vector.tensor_tensor(tmp[:], tmp[:], b2_sb[:],
                                        op=AluOpType.add)
                return tmp

            def l2_out_b(b, tmp):
                nc.sync.dma_start(y.ap()[b * 128:(b + 1) * 128, :], tmp[:])

            chans2 = OUT
            edge_phase(plan.l2, plan.l1.totw, plan.l1.G, tabA2, tabB2,
                       c.ROW2, BF16, 1, OUT, AS2, AS2 + 1,
                       (l2_out_a, l2_out_b), "2", l2_mode=True)

    nc.compile()
    return nc


def host_inputs(cfg: Cfg, plan: EdgePlan, x, W1, att_src1, att_dst1, b1, W2,
                att_src2, att_dst2, b2):
    c = cfg
    H = c.HEADS

    def bf(a):
        import ml_dtypes
        return np.asarray(a, np.float32).astype(ml_dtypes.bfloat16)

    xt = np.zeros((c.IN, c.NPAD), np.float32)
    xt[:, :c.N] = np.asarray(x, np.float32).T
    attbd1 = np.zeros((c.D1, 2 * H), np.float32)
    a_s1 = np.asarray(att_src1, np.float32).reshape(H, c.HID)
    a_d1 = np.asarray(att_dst1, np.float32).reshape(H, c.HID)
    for h in range(H):
        attbd1[h * c.HID:(h + 1) * c.HID, h] = a_s1[h]
        attbd1[h * c.HID:(h + 1) * c.HID, H + h] = a_d1[h]
    att2 = np.stack([np.asarray(att_src2, np.float32).reshape(c.OUT),
                     np.asarray(att_dst2, np.float32).reshape(c.OUT)], axis=1)

    # h1 columns are stored / consumed ch-major (col = ch*H + h): permute
    # everything indexed by D1 accordingly
    W2cm = np.asarray(W2, np.float32).reshape(H, c.HID, c.OUT) \
        .transpose(1, 0, 2).reshape(c.D1, c.OUT)
    b1cm = np.asarray(b1, np.float32).reshape(H, c.HID).T.reshape(c.D1)

    base = {
        "xt": bf(xt),
        "w1": bf(W1),
        "w1t": bf(np.ascontiguousarray(np.asarray(W1, np.float32).T)),
        "attbd1": bf(attbd1),
        "w2": bf(W2cm),
        "w2t": bf(np.ascontiguousarray(W2cm.T)),
        "att2": bf(att2),
        "b1r": np.tile(b1cm.reshape(1, c.D1), (128, 1)),
        "b2r": np.tile(np.asarray(b2, np.float32).reshape(1, c.OUT), (128, 1)),
        "iota_r": bf(np.tile(np.arange(128, dtype=np.float32)[None, :],
                             (128, 1))),
        "iota_c": np.arange(128, dtype=np.float32)[:, None],
        "ident": bf(np.eye(128, dtype=np.float32)),
    }
    in_maps = []
    for k in range(c.NC):
        own_a = 1.0 if (k * c.SHARD) < c.HALF else 0.0
        m = dict(base)
        m["ind_a"] = np.full((128, 1), own_a, np.float32)
        m["ind_b"] = np.full((128, 1), 1.0 - own_a, np.float32)
        m["idx_d"] = np.concatenate([plan.l1.idx[k], plan.l2.idx[k]], axis=1)
        m["dcol_d"] = np.concatenate([plan.l1.dcol[k], plan.l2.dcol[k]],
                                     axis=1)
        in_maps.append(m)
    return in_maps


_CACHE = {}
LAST_RES = None


def kernel(x, edge_index, W1, att_src1, att_dst1, b1, W2, att_src2, att_dst2,
           b2, _cfg=None, _runner=None, _trace=False):
    cfg = _cfg or Cfg()
    ei = np.asarray(edge_index)
    plan = build_edge_plan(cfg, ei)
    key = (cfg.N, cfg.E, cfg.skip_cc, tuple(plan.l1.chA), tuple(plan.l1.chB),
           tuple(plan.l2.chA), tuple(plan.l2.chB))
    if key not in _CACHE:
        _CACHE[key] = build_nc(cfg, plan)
    nc = _CACHE[key]
    in_maps = host_inputs(cfg, plan, x, W1, att_src1, att_dst1, b1, W2,
                          att_src2, att_dst2, b2)
    global LAST_RES
    if _runner is not None:
        results = _runner(nc, in_maps)
    else:
        try:
            res = run_bass_kernel_spmd(nc, in_maps,
                                       core_ids=list(range(cfg.NC)),
                                       trace=_trace)
        except ModuleNotFoundError:
            if not _trace:
                raise
            # NTFF profiling hook unavailable in this environment
            res = run_bass_kernel_spmd(nc, in_maps,
                                       core_ids=list(range(cfg.NC)))
        LAST_RES = res
        results = res.results
    out = np.concatenate([results[k]["y"] for k in range(cfg.NC)], axis=0)
    return np.ascontiguousarray(out[:cfg.N]).astype(np.float32)

